# revision 3
# baseline (speedup 1.0000x reference)
"""ACE loss kernel for 8 Trainium2 NeuronCores (data-parallel over batch rows).

Host: tiny SVD whitening setup (D x D), signature whitening, sharding/layout.
Device (per core, 8192 rows): Y^T = W^T X^T - mu_w, row norms via Gram diag,
ACE = (Y/||Y||) @ sHat^T, logsumexp row sums, label-gather numerator.
Host: assemble ACE, loss = -mean(ln(numexp) - ln(sumexp)).
"""
import sys, types
import numpy as np

# ---------------------------------------------------------------------------
# environment: make concourse importable + register the NTFF profile hook
# ---------------------------------------------------------------------------
if "/opt/trn_rl_repo" not in sys.path:
    sys.path.insert(0, "/opt/trn_rl_repo")

if "antenv.axon_hooks" not in sys.modules:
    _hooks_mod = types.ModuleType("antenv.axon_hooks")
    _hook_holder = {"hook": None}
    _hooks_mod.set_axon_ntff_profile_hook = lambda h: _hook_holder.__setitem__("hook", h)
    _hooks_mod.get_axon_ntff_profile_hook = lambda: _hook_holder["hook"]
    sys.modules["antenv.axon_hooks"] = _hooks_mod
    try:
        from trn_agent_boot.trn_boot import _ntff_profile_via_ctypes
        _hooks_mod.set_axon_ntff_profile_hook(
            _ntff_profile_via_ctypes("/opt/axon/libaxon_pjrt.so"))
    except Exception:
        pass

from contextlib import ExitStack

import concourse.bass as bass
import concourse.tile as tile
from concourse import bacc, mybir
from concourse.bass_utils import run_bass_kernel_spmd

B, C, D = 65536, 1000, 256
N_CORES = 8
B_SHARD = B // N_CORES          # 8192
SUPER = 512                     # rows per supertile
SUB = 128                       # rows per subtile
F32 = mybir.dt.float32
F32R = mybir.dt.float32r
F16 = mybir.dt.float16
BF16 = mybir.dt.bfloat16
AF = mybir.ActivationFunctionType
ALU = mybir.AluOpType


def _r(ap):
    """view a float32 AP as float32r for the tensor engine"""
    return ap.bitcast(F32R)


def build(n_rows=B_SHARD):
    assert n_rows % SUPER == 0
    n_super = n_rows // SUPER
    n_sub = n_rows // SUB
    nc = bacc.Bacc("TRN2", target_bir_lowering=False, debug=False,
                   num_devices=N_CORES)

    xt_d = nc.dram_tensor("xt", [2, 128, n_rows], F32, kind="ExternalInput")
    wsb_d = nc.dram_tensor("wsb", [128, 512], F32, kind="ExternalInput")
    sht_d = nc.dram_tensor("sht", [128, 2 * C], F32, kind="ExternalInput")
    muw_d = nc.dram_tensor("muw", [1, 256], F32, kind="ExternalInput")  # -mu_w
    iota_d = nc.dram_tensor("iota", [128, C], F16, kind="ExternalInput")
    lab_d = nc.dram_tensor("lab", [128, n_sub], F32, kind="ExternalInput")
    dmask_d = nc.dram_tensor("dmask", [128, 512], F32, kind="ExternalInput")
    ones_d = nc.dram_tensor("ones", [1, 512], F32, kind="ExternalInput")
    ace_d = nc.dram_tensor("ace", [n_rows, C], F32, kind="ExternalOutput")
    se_d = nc.dram_tensor("sumexp", [128, n_sub], F32, kind="ExternalOutput")
    ne_d = nc.dram_tensor("numexp", [128, n_sub], F32, kind="ExternalOutput")

    with tile.TileContext(nc) as tc, ExitStack() as ctx:
        cpool = ctx.enter_context(tc.tile_pool(name="const", bufs=1))
        xpool = ctx.enter_context(tc.tile_pool(name="xt", bufs=3))
        ypool = ctx.enter_context(tc.tile_pool(name="yb", bufs=2))
        apool = ctx.enter_context(tc.tile_pool(name="acesb", bufs=3))
        epool = ctx.enter_context(tc.tile_pool(name="expt", bufs=2))
        mpool = ctx.enter_context(tc.tile_pool(name="mask", bufs=2))
        jpool = ctx.enter_context(tc.tile_pool(name="junk", bufs=1))
        spool = ctx.enter_context(tc.tile_pool(name="small", bufs=2))
        yps_pool = ctx.enter_context(tc.tile_pool(name="ypsum", bufs=1, space="PSUM"))
        gps_pool = ctx.enter_context(tc.tile_pool(name="gram", bufs=2, space="PSUM"))
        aps_pool = ctx.enter_context(tc.tile_pool(name="aceps", bufs=2, space="PSUM"))

        # constants (staged f32, rounded to f32r for the tensor engine)
        wsb_s = cpool.tile([128, 512], F32, tag="wsb_s")
        nc.sync.dma_start(wsb_s[:, :], wsb_d.ap()[:, :])
        wsb = cpool.tile([128, 512], F32R, tag="wsb")
        nc.vector.tensor_copy(wsb[:, :], wsb_s[:, :])
        sht_s = cpool.tile([128, 2 * C], F32, tag="sht_s")
        nc.sync.dma_start(sht_s[:, :], sht_d.ap()[:, :])
        sht = cpool.tile([128, 2 * C], F32R, tag="sht")
        nc.vector.tensor_copy(sht[:, :], sht_s[:, :])
        mu_s = cpool.tile([1, 256], F32, tag="mu_s")
        nc.sync.dma_start(mu_s[:, :], muw_d.ap()[:, :])
        mu = cpool.tile([1, 256], F32R, tag="mu")
        nc.vector.tensor_copy(mu[:, :], mu_s[:, :])
        iota = cpool.tile([128, C], F16, tag="iota")
        nc.sync.dma_start(iota[:, :], iota_d.ap()[:, :])
        lab = cpool.tile([128, n_sub], F32, tag="lab")
        nc.sync.dma_start(lab[:, :], lab_d.ap()[:, :])
        dmask = cpool.tile([128, 512], F32, tag="dmask")
        nc.sync.dma_start(dmask[:, :], dmask_d.ap()[:, :])
        ones_s = cpool.tile([1, 512], F32, tag="ones_s")
        nc.sync.dma_start(ones_s[:, :], ones_d.ap()[:, :])
        ones = cpool.tile([1, 512], F32R, tag="ones")
        nc.vector.tensor_copy(ones[:, :], ones_s[:, :])
        se_acc = cpool.tile([128, n_sub], F32, tag="seacc")
        ne_acc = cpool.tile([128, n_sub], F32, tag="neacc")

        for t in range(n_super):
            # ---- load X^T tiles (feature-major), k-chunks of 128 features
            xts = []
            for k in range(2):
                xt_s = xpool.tile([128, SUPER], F32, tag=f"xts{k}")
                nc.sync.dma_start(xt_s[:, :], xt_d.ap()[k, :, t * SUPER:(t + 1) * SUPER])
                xt = xpool.tile([128, SUPER], F32R, tag=f"xt{k}")
                nc.vector.tensor_copy(xt[:, :], xt_s[:, :])
                xts.append(xt)

            # ---- mm1: Y^T [256 feats as 2x128, SUPER rows] = W^T X^T - mu_w
            yps = yps_pool.tile([128, 1024], F32, tag="yps")
            for m in range(2):
                o = yps[:, m * 512:(m + 1) * 512]
                for k in range(2):
                    nc.tensor.matmul(
                        o, lhsT=wsb[:, k * 256 + m * 128: k * 256 + m * 128 + 128],
                        rhs=xts[k][:, :], start=(k == 0), stop=False)
                nc.tensor.matmul(
                    o, lhsT=mu[0:1, m * 128:(m + 1) * 128],
                    rhs=ones[0:1, :], start=False, stop=True)

            # ---- evict Y^T to SBUF (fp32; used as mm2 stationary)
            yb = ypool.tile([128, 1024], F32R, tag="yb")
            nc.vector.tensor_copy(yb[:, 0:512], yps[:, 0:512])
            nc.vector.tensor_copy(yb[:, 512:1024], yps[:, 512:1024])

            # ---- row norms^2 via Gram diagonals (4 subtiles per supertile)
            gram = gps_pool.tile([128, 512], F32, tag="gram")
            for u in range(4):
                for k in range(2):
                    sl = yb[:, k * 512 + u * 128: k * 512 + u * 128 + 128]
                    nc.tensor.matmul(gram[:, u * 128:(u + 1) * 128],
                                     lhsT=sl, rhs=sl,
                                     start=(k == 0), stop=(k == 1))
            n2 = spool.tile([128, 4], F32, tag="n2")
            jg = jpool.tile([128, 128], BF16, tag="jg")
            for u in range(4):
                nc.vector.scalar_tensor_tensor(
                    out=jg[:, :], in0=gram[:, u * 128:(u + 1) * 128], scalar=1.0,
                    in1=dmask[:, u * 128:(u + 1) * 128],
                    op0=ALU.bypass, op1=ALU.mult, accum_out=n2[:, u:u + 1])
            # inv_norm = exp(-0.5 * ln(n2))   (Rsqrt is banned on ScalarE)
            lnn = spool.tile([128, 4], F32, tag="lnn")
            nc.scalar.activation(lnn[:, :], n2[:, :], AF.Ln)
            inv = spool.tile([128, 4], F32, tag="inv")
            nc.scalar.activation(inv[:, :], lnn[:, :], AF.Exp, scale=-0.5)

            # ---- per 128-row subtile: ACE, exp-sum, numerator
            for u in range(4):
                s = t * 4 + u
                aps = aps_pool.tile([128, 1024], F32, tag="aps")
                for k in range(2):
                    sl = yb[:, k * 512 + u * 128: k * 512 + u * 128 + 128]
                    nc.tensor.matmul(aps[:, 0:512], lhsT=sl,
                                     rhs=sht[:, k * C: k * C + 512],
                                     start=(k == 0), stop=(k == 1))
                    nc.tensor.matmul(aps[:, 512:1000], lhsT=sl,
                                     rhs=sht[:, k * C + 512: k * C + C],
                                     start=(k == 0), stop=(k == 1))

                ace = apool.tile([128, C], F32, tag="ace")
                nc.scalar.activation(ace[:, :], aps[:, 0:C], AF.Copy,
                                     scale=inv[:, u:u + 1])
                expt = epool.tile([128, C], F16, tag="expt")
                nc.scalar.activation(expt[:, :], aps[:, 0:C], AF.Exp,
                                     scale=inv[:, u:u + 1],
                                     accum_out=se_acc[:, s:s + 1])
                mask = mpool.tile([128, C], F16, tag="mask")
                nc.vector.tensor_scalar(out=mask[:, :], in0=iota[:, :],
                                        scalar1=lab[:, s:s + 1], scalar2=None,
                                        op0=ALU.is_equal)
                junk = jpool.tile([128, C], F16, tag="junk")
                nc.vector.scalar_tensor_tensor(
                    out=junk[:, :], in0=expt[:, :], scalar=1.0, in1=mask[:, :],
                    op0=ALU.bypass, op1=ALU.mult, accum_out=ne_acc[:, s:s + 1])

                nc.sync.dma_start(ace_d.ap()[t * SUPER + u * SUB: t * SUPER + (u + 1) * SUB, :],
                                  ace[:, :])

        nc.sync.dma_start(se_d.ap()[:, :], se_acc[:, :])
        nc.sync.dma_start(ne_d.ap()[:, :], ne_acc[:, :])

    nc.compile()
    return nc


def host_prep(X, labels, signatures, b_means, b_covs, n_rows=B_SHARD):
    """Returns (in_maps, aux) for run_bass_kernel_spmd."""
    X = np.asarray(X, dtype=np.float32)
    labels = np.asarray(labels).astype(np.int32)
    signatures = np.asarray(signatures, dtype=np.float32)
    b_means = np.asarray(b_means, dtype=np.float32)
    b_covs = np.asarray(b_covs, dtype=np.float32)

    cov = b_covs @ b_covs.T
    U, eig, _ = np.linalg.svd(cov)
    DU = (eig ** -0.5)[:, None] * U.T          # [D, D]
    W = np.ascontiguousarray(DU.T)             # Y = (X - mu) @ W
    mu_w = (b_means @ W).reshape(1, D)         # [1, D]
    s_w = signatures @ W
    sHat = s_w / np.maximum(np.linalg.norm(s_w, axis=1, keepdims=True), 1e-12)
    SHT = np.ascontiguousarray(sHat.T)         # [D, C]

    wsb = np.ascontiguousarray(
        W.reshape(2, 128, 256).transpose(1, 0, 2).reshape(128, 512))
    sht = np.ascontiguousarray(
        SHT.reshape(2, 128, C).transpose(1, 0, 2).reshape(128, 2 * C))
    muw = np.ascontiguousarray(-mu_w)
    iota = np.ascontiguousarray(
        np.tile(np.arange(C, dtype=np.float16)[None, :], (128, 1)))
    dmask = np.ascontiguousarray(np.tile(np.eye(128, dtype=np.float32), (1, 4)))

    n_sub = n_rows // SUB
    in_maps = []
    for i in range(N_CORES):
        xs = X[i * n_rows:(i + 1) * n_rows]
        ls = labels[i * n_rows:(i + 1) * n_rows]
        xt = np.ascontiguousarray(xs.T).reshape(2, 128, n_rows)
        lab = np.ascontiguousarray(ls.reshape(n_sub, 128).T.astype(np.float32))
        in_maps.append({"xt": xt, "wsb": wsb, "sht": sht, "muw": muw,
                        "iota": iota, "lab": lab, "dmask": dmask,
                        "ones": np.ones((1, 512), np.float32)})
    return in_maps


_CACHE = {}


def run_device(X, labels, signatures, b_means, b_covs, n_rows=B_SHARD,
               trace=False):
    if n_rows not in _CACHE:
        _CACHE[n_rows] = build(n_rows)
    nc = _CACHE[n_rows]
    in_maps = host_prep(X, labels, signatures, b_means, b_covs, n_rows)
    res = run_bass_kernel_spmd(nc, in_maps, core_ids=list(range(N_CORES)),
                               trace=trace)
    return res


def finish(res, n_rows=B_SHARD):
    n_used = N_CORES * n_rows
    ace = np.concatenate([res.results[i]["ace"] for i in range(N_CORES)], axis=0)
    se = np.concatenate(
        [res.results[i]["sumexp"].flatten(order="F") for i in range(N_CORES)])
    ne = np.concatenate(
        [res.results[i]["numexp"].flatten(order="F") for i in range(N_CORES)])
    loss = -np.mean(np.log(ne.astype(np.float64))
                    - np.log(se.astype(np.float64)))
    return np.float32(loss), ace


def kernel(X, labels, signatures, b_means, b_covs):
    res = run_device(X, labels, signatures, b_means, b_covs)
    loss, ace = finish(res)
    return loss, ace
